# revision 34
# baseline (speedup 1.0000x reference)
"""Bass/Trainium2 kernel for batched masked-Kabsch RMSD (nn_Coords2RMSD).

Strategy (per NeuronCore, SPMD across 8 cores):
  - Host sorts batch rows by num_atoms into 4 size classes (quartiles); core c
    takes one 128-row tile from each class, capped at the class max atom count
    rounded to 256. Padded atoms are zeroed on the host so they drop out of
    every statistic.
  - Data is uploaded fp8-e4m3 in a transposed, pair-interleaved layout
    ([atom-in-chunk partitions] x [chunk, component, (row, ktile) pairs],
    column-reversed per the DoubleRowSwInterleave weight format, with a ones
    column per component block).
  - The TensorEngine computes, per class, Gram blocks accumulated in PSUM via
    fp8 DoubleRowSwInterleave matmuls (256 atoms per instruction):
      out_j = Y_j^T [X_0|1|X_1|1|X_2|1]  -> R_ij on block diagonals, Sy_j cols
      gx_i  = X_i^T [X_i|1]              -> |x_i|^2 diagonal, Sx_i col
      gy_j  = Y_j^T [Y_j|1]              -> |y_j|^2 diagonal, Sy_j col
  - Act evacuates PSUM to SBUF f32; DVE extracts the diagonals with masked
    scalar_tensor_tensor accumulations (identity / triple-shifted-diag masks).
  - Final stage (tiny [128, 4] fp32 tiles): centroid corrections, 3x3 C^T C
    eigenvalues via the closed-form trigonometric method, Kabsch det sign,
    RMSD.
"""

import numpy as np

import concourse.bass as bass
import concourse.mybir as mybir
from concourse.tile import TileContext, ScopedClock

F32 = mybir.dt.float32
FP8 = mybir.dt.float8e4
OP = mybir.AluOpType
AF = mybir.ActivationFunctionType
MM = mybir.MatmulPerfMode

N_CORES = 8
ROWS = 128      # rows per tile == SBUF partitions == matmul stationary cols
CHUNK = 256     # atoms per DoubleRow matmul
BLK = 258       # fp8 cols per component block: (128 rows + 1 ones) * 2 ktiles
CW = 3 * BLK    # per-chunk width (3 components)
GW = 390        # gx/gy psum width: 3 blocks of 129 at 130 spacing
JW = 387        # j-gram width: 3 blocks of 129
CPW = 3 * JW + 2 * GW  # per-class copy width (1941)


# ---------------------------------------------------------------------------
# TileContext tail patch: this walrus build accepts at most ONE sync-wait
# command per instruction and no sem-eq waits, so the stock drain + EVSEM
# butterfly fails codegen. Emit a ge-wait-only tail instead.
# ---------------------------------------------------------------------------
def _patched_drain_and_barrier(self, tick_clock, wait_clock):
    nc = self.nc
    dummy = nc.gpsimd.nop()
    wait_clock.add_sem_waits(dummy.ins, ScopedClock({None: tick_clock.global_clock}))
    waits = list(dummy.ins.sync_info.on_wait) if dummy.ins.sync_info else []
    if dummy.ins.sync_info:
        dummy.ins.sync_info = mybir.SyncInfo(on_wait=[], on_update=[])

    bsem = nc.alloc_semaphore(f"tail_bsem_{nc.next_id()}")
    dsem = nc.alloc_semaphore(f"tail_dsem_{nc.next_id()}")
    n_eng = 0
    for eng in nc.engines.values():
        eng.drain()
        eng.sem_inc(bsem, 1)
        n_eng += 1
    nc.gpsimd.wait_ge(bsem, n_eng)
    for w in waits:
        n = nc.gpsimd.nop()
        n.ins.sync_info = mybir.SyncInfo(on_wait=[w], on_update=[])
    nc.gpsimd.sem_inc(dsem, 1)
    for eng in nc.engines.values():
        if eng is not nc.gpsimd:
            eng.wait_ge(dsem, 1)

    popped = nc._tile_sem_poison_stack.pop()
    assert popped is self._sem_poison
    nc.clear_and_free_semaphores(list(self.sems.allocated().values()))
    nc.gpsimd.sem_clear(bsem)
    nc.gpsimd.sem_clear(dsem)


def install_tile_patch():
    TileContext._drain_and_barrier = _patched_drain_and_barrier


# ---------------------------------------------------------------------------
# BIR post-pass: this walrus build accepts at most one sync-wait command per
# instruction (none on Drain). Tile's sem-assigner can attach several, so
# split extras onto same-engine NoOps inserted just before the instruction.
# ---------------------------------------------------------------------------
_orig_to_json_bytes = bass.Bass.to_json_bytes


def _split_multiwait_json(self) -> bytes:
    import json

    raw = _orig_to_json_bytes(self)
    m = json.loads(raw)
    ctr = 0
    changed = False
    for f in m.get("functions", []):
        for blk in f.get("blocks", []):
            insts = blk.get("instructions", [])
            out = []
            for inst in insts:
                si = inst.get("sync_info")
                ow = (si or {}).get("on_wait") or []
                opc = str(inst.get("opcode", inst.get("type", "")))
                limit = 0 if opc == "Drain" else 1
                if len(ow) > limit:
                    keep = ow[len(ow) - limit :] if limit else []
                    moved = ow[: len(ow) - limit] if limit else ow
                    for w in moved:
                        ctr += 1
                        out.append(
                            {
                                "debug": inst.get("debug", 0),
                                "engine": inst["engine"],
                                "ins": [],
                                "name": f"WS-{ctr}-{inst['name']}",
                                "opcode": "NoOp",
                                "outs": [],
                                "sync_info": {"on_update": [], "on_wait": [w]},
                            }
                        )
                    si["on_wait"] = keep
                    changed = True
                out.append(inst)
            blk["instructions"] = out
    if not changed:
        return raw
    return json.dumps(m).encode()


bass.Bass.to_json_bytes = _split_multiwait_json


# ---------------------------------------------------------------------------
# Final math emitter: batched wide fp32 ops, split across DVE/Act/Pool.
# ---------------------------------------------------------------------------
class _FM:
    def __init__(self, nc, pool, K):
        self.nc = nc
        self.pool = pool
        self.K = K
        self.n = 0
        self._consts = {}

    def const_col(self, val):
        val = float(val)
        if val in self._consts:
            return self._consts[val]
        i = len(self._consts)
        t = self.pool.tile([ROWS, 1], F32, tag=f"fmc{i}", name=f"fmc{i}")
        self.nc.vector.memset(t[:], val)
        self._consts[val] = t[:]
        return t[:]

    def t(self, w=None):
        self.n += 1
        w = w or self.K
        return self.pool.tile([ROWS, w], F32, tag=f"fm{self.n}", name=f"fm{self.n}")

    def tt(self, a, b, op, w=None, eng=None):
        o = self.t(w)
        (eng or self.nc.vector).tensor_tensor(o[:], a, b, op)
        return o[:]

    def mul(self, a, b, w=None, eng=None):
        o = self.t(w)
        eng = eng or self.nc.vector
        if eng is self.nc.gpsimd:
            eng.tensor_mul(o[:], a, b)
        else:
            eng.tensor_tensor(o[:], a, b, OP.mult)
        return o[:]

    def add(self, a, b, w=None, eng=None):
        o = self.t(w)
        eng = eng or self.nc.vector
        if eng is self.nc.gpsimd:
            eng.tensor_add(o[:], a, b)
        else:
            eng.tensor_tensor(o[:], a, b, OP.add)
        return o[:]

    def sub(self, a, b, w=None, eng=None):
        o = self.t(w)
        eng = eng or self.nc.vector
        if eng is self.nc.gpsimd:
            eng.tensor_sub(o[:], a, b)
        else:
            eng.tensor_tensor(o[:], a, b, OP.subtract)
        return o[:]

    def ts(self, a, s, op, w=None):
        o = self.t(w)
        self.nc.vector.tensor_scalar(o[:], a, float(s), None, op)
        return o[:]

    def ts2(self, a, s1, s2, op0, op1):
        o = self.t()
        self.nc.vector.tensor_scalar(o[:], a, float(s1), float(s2), op0, op1)
        return o[:]

    def stt(self, a, s, b, op0, op1, w=None):
        """(a op0 s) op1 b"""
        o = self.t(w)
        self.nc.vector.scalar_tensor_tensor(o[:], a, float(s), b, op0, op1)
        return o[:]

    def act(self, a, func, bias=0.0, scale=1.0, w=None):
        o = self.t(w)
        if isinstance(bias, float) and bias not in (0.0, 1.0) and func != AF.Copy:
            bias = self.const_col(bias)
        self.nc.scalar.activation(o[:], a, func, bias=bias, scale=scale)
        return o[:]

    def recip(self, a):
        o = self.t()
        self.nc.vector.reciprocal(o[:], a)
        return o[:]


def _emit_final_math(nc, fm, st_rxy, st_g, sxp, syp, n_ap, out_ap):
    """st_rxy: [128, 9K] raw moments, col (3i+j)K+k; st_g: [128, K] (Gx+Gy);
    sxp/syp: [128, 3K] packed raw sums, col iK+k; n_ap: [128, K]."""
    K = fm.K
    pool_eng = nc.gpsimd
    rn = fm.recip(n_ap)

    # C = st_rxy - (Sx_i Sy_j) rn : three wide [128, 9K] ops
    sxA = sxp.rearrange("p (i k) -> p i k", i=3).unsqueeze(2).broadcast_to([ROWS, 3, 3, K])
    syB = syp.rearrange("p (j k) -> p j k", j=3).unsqueeze(1).broadcast_to([ROWS, 3, 3, K])
    t1 = fm.t(9 * K)
    nc.vector.tensor_tensor(t1[:].rearrange("p (i j k) -> p i j k", i=3, j=3), sxA, syB, OP.mult)
    rn9 = rn.unsqueeze(1).broadcast_to([ROWS, 9, K])
    t2 = fm.t(9 * K)
    nc.vector.tensor_tensor(t2[:].rearrange("p (g k) -> p g k", g=9), t1[:].rearrange("p (g k) -> p g k", g=9), rn9, OP.mult)
    Ct = fm.sub(st_rxy, t2[:], w=9 * K)
    Cr = Ct.rearrange("p (i j k) -> p i j k", i=3, j=3)

    def C(i, j):
        return Ct[:, (3 * i + j) * K : (3 * i + j + 1) * K]

    # gxy = (Gx + Gy raw, pre-merged in st_g) - (|Sx|^2 + |Sy|^2) rn  (Pool)
    sqx = fm.mul(sxp, sxp, w=3 * K, eng=pool_eng)
    sqy = fm.mul(syp, syp, w=3 * K, eng=pool_eng)
    sq1 = fm.add(sqx, sqy, w=3 * K, eng=pool_eng)
    sq2 = fm.add(sq1[:, 0:K], sq1[:, K : 2 * K], eng=pool_eng)
    sq3 = fm.add(sq2, sq1[:, 2 * K : 3 * K], eng=pool_eng)
    sqrn = fm.mul(sq3, rn, eng=pool_eng)
    gxy = fm.sub(st_g, sqrn, eng=pool_eng)

    # K = C^T C packed into kkp [128, 6K]: order [00, 11, 22, 01, 02, 12]
    kkp = fm.t(6 * K)
    for idx, (a, b) in enumerate([(0, 0), (1, 1), (2, 2), (0, 1), (0, 2), (1, 2)]):
        m = fm.t(3 * K)
        nc.vector.tensor_tensor(
            m[:].rearrange("p (i k) -> p i k", i=3), Cr[:, :, a, :], Cr[:, :, b, :], OP.mult
        )
        f1 = fm.add(m[:, 0:K], m[:, K : 2 * K])
        nc.vector.tensor_tensor(
            kkp[:, idx * K : (idx + 1) * K], f1, m[:, 2 * K : 3 * K], OP.add
        )
    kdiag = kkp[:, 0 : 3 * K]
    koff = kkp[:, 3 * K : 6 * K]
    k01 = kkp[:, 3 * K : 4 * K]
    k02 = kkp[:, 4 * K : 5 * K]
    k12 = kkp[:, 5 * K : 6 * K]

    # det(C)  (Pool branch; only feeds the Kabsch sign)
    m0 = fm.sub(fm.mul(C(1, 1), C(2, 2), eng=pool_eng), fm.mul(C(1, 2), C(2, 1), eng=pool_eng), eng=pool_eng)
    m1 = fm.sub(fm.mul(C(1, 0), C(2, 2), eng=pool_eng), fm.mul(C(1, 2), C(2, 0), eng=pool_eng), eng=pool_eng)
    m2 = fm.sub(fm.mul(C(1, 0), C(2, 1), eng=pool_eng), fm.mul(C(1, 1), C(2, 0), eng=pool_eng), eng=pool_eng)
    d0 = fm.mul(C(0, 0), m0, eng=pool_eng)
    d1 = fm.mul(C(0, 1), m1, eng=pool_eng)
    d2 = fm.mul(C(0, 2), m2, eng=pool_eng)
    detC = fm.add(fm.sub(d0, d1, eng=pool_eng), d2, eng=pool_eng)
    # Kabsch sign d = 1 - 2*(detC < 0)  (stay on Pool-adjacent DVE ops)
    neg = fm.ts(detC, 0.0, OP.is_lt)
    dsg = fm.ts2(neg, -2.0, 1.0, OP.mult, OP.add)

    # q = tr(K)/3 ; kd = diag - q (one wide op); p2 = |kd|^2 + 2 |koff|^2
    trk = fm.add(fm.add(kkp[:, 0:K], kkp[:, K : 2 * K]), kkp[:, 2 * K : 3 * K])
    q = fm.ts(trk, 1.0 / 3.0, OP.mult)
    kdp = fm.t(3 * K)
    q3 = q.unsqueeze(1).broadcast_to([ROWS, 3, K])
    nc.vector.tensor_tensor(
        kdp[:].rearrange("p (i k) -> p i k", i=3),
        kdiag.rearrange("p (i k) -> p i k", i=3), q3, OP.subtract,
    )
    sq_all = fm.t(6 * K)
    nc.vector.tensor_tensor(sq_all[:], kkp[:], kkp[:], OP.mult)  # squares of all 6
    # p2 = sum over diag squares... need kd squares, not kk squares; redo:
    kdsq = fm.mul(kdp[:], kdp[:], w=3 * K)
    p2a = fm.add(fm.add(kdsq[:, 0:K], kdsq[:, K : 2 * K]), kdsq[:, 2 * K : 3 * K])
    xsq = fm.add(
        fm.add(sq_all[:, 3 * K : 4 * K], sq_all[:, 4 * K : 5 * K]),
        sq_all[:, 5 * K : 6 * K],
    )
    p2 = fm.stt(xsq, 2.0, p2a, OP.mult, OP.add)  # p2a + 2*xsq
    p2c = fm.ts2(p2, 1.0 / 6.0, 1e-30, OP.mult, OP.max)
    p = fm.act(p2c, AF.Sqrt)

    # det(K - qI) (symmetric)
    e0 = fm.mul(kdp[:, 0:K], fm.sub(fm.mul(kdp[:, K : 2 * K], kdp[:, 2 * K : 3 * K]), sq_all[:, 5 * K : 6 * K]))
    e1 = fm.mul(k01, fm.sub(fm.mul(k01, kdp[:, 2 * K : 3 * K]), fm.mul(k12, k02)))
    e2 = fm.mul(k02, fm.sub(fm.mul(k01, k12), fm.mul(kdp[:, K : 2 * K], k02)))
    detKq = fm.add(fm.sub(e0, e1), e2)

    # r = 0.5 det(K-qI) / p^3, clamped to [-1, 1]
    rp = fm.recip(p)
    r = fm.mul(fm.mul(fm.ts(detKq, 0.5, OP.mult), rp), fm.mul(rp, rp))
    r = fm.ts2(r, 1.0, -1.0, OP.min, OP.max)

    # acos via |r| fold:
    #   A = 2 atan(sqrt((1-|r|)/(1+|r|))) = acos(|r|)
    #   acos(r) = A + (r<0) * (pi - 2A) ; phi = acos(r)/3
    rabs = fm.stt(r, -1.0, r, OP.mult, OP.max)  # |r| = max(-r, r)
    onemr = fm.ts2(rabs, -1.0, 1.0, OP.mult, OP.add)  # 1 - |r|
    onepr = fm.ts(rabs, 1.0, OP.add)
    u = fm.mul(onemr, fm.recip(onepr))
    su = fm.act(u, AF.Sqrt)
    at = fm.act(su, AF.Arctan)  # Act->Act, single round trip
    A = fm.ts(at, 2.0, OP.mult)
    rneg = fm.ts(r, 0.0, OP.is_lt)
    corr = fm.ts2(A, -2.0, float(np.pi), OP.mult, OP.add)  # pi - 2A
    acr = fm.add(A, fm.mul(rneg, corr))
    # cos(phi) = sin(phi/1 + pi/2) ; cos(phi + 2pi/3) = -sin(5pi/6 - phi)
    # pack both sin args into one [128, 2K] tile -> single Act Sin
    sargs = fm.t(2 * K)
    nc.vector.tensor_scalar(sargs[:, 0:K], acr, 1.0 / 3.0, float(np.pi / 2), OP.mult, OP.add)
    nc.vector.tensor_scalar(sargs[:, K : 2 * K], acr, -1.0 / 3.0, float(5 * np.pi / 6), OP.mult, OP.add)
    sc = fm.act(sargs[:], AF.Sin, w=2 * K)

    # eigenvalues packed: l = [l1 | l2 | l3], one clamp + one sqrt
    p2x = fm.ts(p, 2.0, OP.mult)
    lp = fm.t(3 * K)
    l1 = fm.add(q, fm.mul(p2x, sc[:, 0:K]))
    l3 = fm.sub(q, fm.mul(p2x, sc[:, K : 2 * K]))
    nc.vector.tensor_copy(lp[:, 0:K], l1)
    nc.vector.tensor_copy(lp[:, 2 * K : 3 * K], l3)
    nc.vector.tensor_tensor(
        lp[:, K : 2 * K], fm.stt(q, 3.0, l1, OP.mult, OP.subtract), l3, OP.subtract
    )
    lc = fm.ts(lp[:], 0.0, OP.max, w=3 * K)
    sv = fm.act(lc, AF.Sqrt, w=3 * K)

    tr = fm.add(fm.add(sv[:, 0:K], sv[:, K : 2 * K]), fm.mul(dsg, sv[:, 2 * K : 3 * K]))

    # msd = (gxy - 2 tr) rn ; rmsd = sqrt(max(msd, 0))
    diff = fm.stt(tr, -2.0, gxy, OP.mult, OP.add)
    msd = fm.mul(diff, rn)
    rmsd = fm.act(fm.ts(msd, 0.0, OP.max), AF.Sqrt)
    nc.vector.tensor_copy(out_ap, rmsd)


# ---------------------------------------------------------------------------
# Program builder
# ---------------------------------------------------------------------------
def build_program(caps, nmax, cfg=None):
    """caps: per-class atom capacities (multiples of 256). Returns nc."""
    cfg = cfg or {}
    K = len(caps)
    chunks = [c // CHUNK for c in caps]
    assert all(c * CHUNK == cap for c, cap in zip(chunks, caps))
    offs = np.cumsum([0] + [c * CW for c in chunks])
    TOTW = int(offs[-1])

    install_tile_patch()
    nc = bass.Bass()
    x_d = nc.dram_tensor("x", [ROWS, TOTW], FP8, kind="ExternalInput")
    y_d = nc.dram_tensor("y", [ROWS, TOTW], FP8, kind="ExternalInput")
    # consts packed: [meta (K) | imask (128) | gmask (780)]
    CONW = K + ROWS + 2 * GW
    consts_d = nc.dram_tensor("consts", [ROWS, CONW], F32, kind="ExternalInput")
    out_d = nc.dram_tensor("out", [ROWS, K], F32, kind="ExternalOutput")

    with TileContext(nc) as tc:
        with (
            tc.tile_pool(name="const", bufs=1) as constp,
            tc.tile_pool(name="data", bufs=1) as datap,
            tc.tile_pool(name="stats", bufs=1) as statp,
            tc.tile_pool(name="cps", bufs=2) as cpsp,
            tc.psum_pool(name="pj", bufs=2) as pjp,
            tc.psum_pool(name="pg", bufs=1) as pgp,
        ):
            consts_t = constp.tile([ROWS, CONW], F32)
            nc.sync.dma_start(out=consts_t[:], in_=consts_d[:])
            meta_t = consts_t[:, 0:K]
            imask_t = consts_t[:, K : K + ROWS]
            gmask_t = consts_t[:, K + ROWS : CONW]

            # Largest class first: its transfer + matmuls dominate, and the
            # last class on the critical tail is then the smallest.
            korder = list(range(K))[::-1]
            x_t = datap.tile([ROWS, TOTW], FP8, name="x_t", tag="x_t")
            y_t = datap.tile([ROWS, TOTW], FP8, name="y_t", tag="y_t")
            for k in korder:
                a, b = int(offs[k]), int(offs[k + 1])
                nc.sync.dma_start(out=x_t[:, a:b], in_=x_d[:, a:b])
                nc.sync.dma_start(out=y_t[:, a:b], in_=y_d[:, a:b])

            st_rxy = statp.tile([ROWS, 9 * K], F32)
            st_g = statp.tile([ROWS, K], F32)
            sxp = statp.tile([ROWS, 3 * K], F32, name="sxp", tag="sxp")
            syp = statp.tile([ROWS, 3 * K], F32, name="syp", tag="syp")
            scr = statp.tile([ROWS, 2 * GW], F32, name="scr", tag="scr")
            phase = cfg.get("phase", "full")  # dma|mm|copies|extract|full

            for k in korder:
                if phase == "dma":
                    break
                base_k = int(offs[k])
                nck = chunks[k]
                oj = [
                    pjp.tile([ROWS, JW], F32, name=f"oj{j}", tag=f"oj{j}")
                    for j in range(3)
                ]
                ogx = pgp.tile([ROWS, GW], F32, name="ogx", tag="ogx")
                ogy = pgp.tile([ROWS, GW], F32, name="ogy", tag="ogy")
                # Multiple accumulation groups share the ogx/ogy banks and
                # start=True zeroes a whole bank, so only the very first
                # matmul of each bank carries start=True.

                for c in range(nck):
                    st = c == 0
                    sp = c == nck - 1
                    cb = base_k + c * CW
                    rhs_x = x_t[:, cb : cb + CW].rearrange(
                        "p (b c2 two) -> p two b c2", two=2, b=3
                    )[:, :, :, ::-1]
                    for j in range(3):
                        lhsT = y_t[:, cb + BLK * j + 2 : cb + BLK * (j + 1)]
                        nc.tensor.matmul(
                            oj[j][:], lhsT, rhs_x, start=st, stop=sp,
                            perf_mode=MM.DoubleRowSwInterleave,
                        )
                    for i in range(3):
                        sl = slice(cb + BLK * i, cb + BLK * (i + 1))
                        rhs_xi = x_t[:, sl].rearrange(
                            "p (c2 two) -> p two c2", two=2
                        )[:, :, ::-1]
                        nc.tensor.matmul(
                            ogx[:, 130 * i : 130 * i + 129],
                            x_t[:, sl][:, 2:], rhs_xi,
                            start=(st and i == 0), stop=sp,
                            perf_mode=MM.DoubleRowSwInterleave,
                            skip_group_check=True,
                        )
                    for j in range(3):
                        sl = slice(cb + BLK * j, cb + BLK * (j + 1))
                        rhs_yj = y_t[:, sl].rearrange(
                            "p (c2 two) -> p two c2", two=2
                        )[:, :, ::-1]
                        nc.tensor.matmul(
                            ogy[:, 130 * j : 130 * j + 129],
                            y_t[:, sl][:, 2:], rhs_yj,
                            start=(st and j == 0), stop=sp,
                            perf_mode=MM.DoubleRowSwInterleave,
                            skip_group_check=True,
                        )

                if phase == "mm":
                    continue
                # Act: evacuate PSUM -> SBUF f32, per-class tiles (bufs=2 so
                # class k+1 copies overlap class k extraction on DVE).
                cj = [
                    cpsp.tile([ROWS, JW], F32, name=f"cj{j}", tag=f"cj{j}")
                    for j in range(3)
                ]
                cg = cpsp.tile([ROWS, 2 * GW], F32, name="cg", tag="cg")
                for j in range(3):
                    nc.scalar.copy(cj[j][:], oj[j][:])
                nc.scalar.copy(cg[:, 0:GW], ogx[:])
                nc.scalar.copy(cg[:, GW : 2 * GW], ogy[:])

                if phase == "copies":
                    continue
                # DVE: masked diagonal extraction
                # st_rxy layout: col (3i+j)*K + k ; st_g: Gx+Gy at col k
                for j in range(3):
                    for i in range(3):
                        col = (3 * i + j) * K + k
                        nc.vector.scalar_tensor_tensor(
                            scr[:, 0:ROWS],
                            cj[j][:, 129 * i : 129 * i + ROWS],
                            1.0, imask_t, OP.mult, OP.mult,
                            accum_out=st_rxy[:, col : col + 1],
                        )
                nc.vector.scalar_tensor_tensor(
                    scr[:], cg[:], 1.0, gmask_t, OP.mult, OP.mult,
                    accum_out=st_g[:, k : k + 1],
                )
                # Raw sums from the ones-columns (cols 128::130 of each bank):
                # sxp/syp [128, 3K], col i*K + k.
                nc.vector.tensor_copy(sxp[:, k : 2 * K + k + 1 : K], cg[:, 128:GW:130])
                nc.vector.tensor_copy(syp[:, k : 2 * K + k + 1 : K], cg[:, GW + 128 : 2 * GW : 130])

            out_t = statp.tile([ROWS, K], F32)
            if phase == "full":
                fm = _FM(nc, statp, K)
                _emit_final_math(
                    nc, fm, st_rxy[:], st_g[:], sxp[:], syp[:], meta_t, out_t[:]
                )
            else:
                nc.vector.memset(out_t[:], 0.0)
            nc.sync.dma_start(out=out_d[:], in_=out_t[:])

            if cfg.get("debug"):
                dbg_rxy = nc.dram_tensor("dbg_rxy", [ROWS, 9 * K], F32, kind="ExternalOutput")
                dbg_g = nc.dram_tensor("dbg_g", [ROWS, K], F32, kind="ExternalOutput")
                dbg_sx = nc.dram_tensor("dbg_sx", [ROWS, 3 * K], F32, kind="ExternalOutput")
                dbg_sy = nc.dram_tensor("dbg_sy", [ROWS, 3 * K], F32, kind="ExternalOutput")
                nc.sync.dma_start(out=dbg_rxy[:], in_=st_rxy[:])
                nc.sync.dma_start(out=dbg_g[:], in_=st_g[:])
                nc.sync.dma_start(out=dbg_sx[:], in_=sxp[:])
                nc.sync.dma_start(out=dbg_sy[:], in_=syp[:])

    return nc


# ---------------------------------------------------------------------------
# Host side
# ---------------------------------------------------------------------------
def plan_shards(num_atoms, n_classes=4, cap_round=CHUNK):
    B = num_atoms.shape[0]
    assert B % (N_CORES * ROWS) == 0
    n_classes_total = B // (N_CORES * ROWS)
    assert n_classes == n_classes_total
    order = np.argsort(num_atoms, kind="stable")
    na_sorted = num_atoms[order]
    rows_per_class = N_CORES * ROWS
    caps = []
    for k in range(n_classes):
        mx = int(na_sorted[(k + 1) * rows_per_class - 1])
        cap = ((mx + cap_round - 1) // cap_round) * cap_round
        caps.append(cap)
    return order, caps


def shard_inputs(coords_input, coords_target, num_atoms, order, caps, nmax):
    import ml_dtypes

    K = len(caps)
    rows_per_class = N_CORES * ROWS
    chunks = [c // CHUNK for c in caps]
    TOTW = sum(c * CW for c in chunks)

    imask = np.eye(ROWS, dtype=np.float32)
    gmask = np.zeros((ROWS, 2 * GW), dtype=np.float32)
    r = np.arange(ROWS)
    for i in range(3):
        gmask[r, 130 * i + r] = 1.0
        gmask[r, GW + 130 * i + r] = 1.0

    in_maps = []
    core_row_idx = []
    for c in range(N_CORES):
        idx = np.concatenate(
            [
                order[k * rows_per_class + c * ROWS : k * rows_per_class + (c + 1) * ROWS]
                for k in range(K)
            ]
        )
        core_row_idx.append(idx)
        na_c = num_atoms[idx].astype(np.int64)  # [K*ROWS]

        bufs = {"x": np.zeros((ROWS, TOTW), dtype=ml_dtypes.float8_e4m3fn),
                "y": np.zeros((ROWS, TOTW), dtype=ml_dtypes.float8_e4m3fn)}
        off = 0
        for k in range(K):
            rows_k = idx[k * ROWS : (k + 1) * ROWS]
            na_k = na_c[k * ROWS : (k + 1) * ROWS]
            cap = caps[k]
            nck = chunks[k]
            amask = (np.arange(cap)[None, :] < na_k[:, None])
            for nm, coords in (("x", coords_input), ("y", coords_target)):
                v = coords[rows_k, : 3 * cap].reshape(ROWS, cap, 3)
                v = np.where(amask[:, :, None], v, 0.0).astype(np.float32)
                # [r, ch, t, p, i] -> [p, ch, i, r, t]
                v = v.reshape(ROWS, nck, 2, 128, 3).transpose(3, 1, 4, 0, 2)
                blk = np.zeros((128, nck, 3, 129, 2), dtype=ml_dtypes.float8_e4m3fn)
                blk[:, :, :, 1:, :] = v[:, :, :, ::-1, :].astype(
                    ml_dtypes.float8_e4m3fn
                )
                blk[:, :, :, 0, :] = 1.0
                w = nck * CW
                bufs[nm][:, off : off + w] = blk.reshape(128, w)
            off += nck * CW

        meta = np.ascontiguousarray(na_c.astype(np.float32).reshape(K, ROWS).T)
        consts = np.concatenate([meta, imask, gmask], axis=1)
        in_maps.append({"x": bufs["x"], "y": bufs["y"], "consts": consts})
    return in_maps, core_row_idx


def unshard_outputs(results, core_row_idx, B):
    out = np.empty(B, dtype=np.float32)
    for c in range(N_CORES):
        o = results[c]["out"]  # [ROWS, K]
        idx = core_row_idx[c]
        out[idx] = o.T.reshape(-1)
    return out


# ---------------------------------------------------------------------------
# Entry point: full inputs in, full output out. Shards across 8 NeuronCores.
# ---------------------------------------------------------------------------
_PROG_CACHE = {}


def _get_program(caps, nmax):
    key = (tuple(caps), nmax)
    if key not in _PROG_CACHE:
        _PROG_CACHE[key] = build_program(list(caps), nmax)
    return _PROG_CACHE[key]


def kernel(coords_input, coords_target, num_atoms):
    from concourse.bass_utils import run_bass_kernel_spmd

    x = np.ascontiguousarray(np.asarray(coords_input, dtype=np.float32))
    y = np.ascontiguousarray(np.asarray(coords_target, dtype=np.float32))
    na = np.asarray(num_atoms)
    na_i = na.astype(np.int64)
    B, ncols = x.shape
    nmax = ncols // 3
    K = B // (N_CORES * ROWS)
    assert B == N_CORES * ROWS * K, f"unsupported batch {B}"

    order, caps = plan_shards(na_i, n_classes=K)
    in_maps, core_row_idx = shard_inputs(x, y, na_i, order, caps, nmax)
    nc = _get_program(caps, nmax)
    res = run_bass_kernel_spmd(nc, in_maps, core_ids=list(range(N_CORES)))
    out = unshard_outputs(res.results, core_row_idx, B)
    return out.astype(np.float32)


# revision 37
# speedup vs baseline: 1.1761x; 1.1761x over previous
"""Bass/Trainium2 kernel for batched masked-Kabsch RMSD (nn_Coords2RMSD).

Strategy (per NeuronCore, SPMD across 8 cores):
  - Host sorts batch rows by num_atoms into 4 size classes (quartiles); core c
    takes one 128-row tile from each class, capped at the class max atom count
    rounded to 256. Padded atoms are zeroed on the host so they drop out of
    every statistic.
  - Data is uploaded fp8-e4m3 in a transposed, pair-interleaved layout
    ([atom-in-chunk partitions] x [chunk, component, (row, ktile) pairs],
    column-reversed per the DoubleRowSwInterleave weight format, with a ones
    column per component block).
  - The TensorEngine computes, per class, Gram blocks accumulated in PSUM via
    fp8 DoubleRowSwInterleave matmuls (256 atoms per instruction):
      out_j = Y_j^T [X_0|1|X_1|1|X_2|1]  -> R_ij on block diagonals, Sy_j cols
      gx_i  = X_i^T [X_i|1]              -> |x_i|^2 diagonal, Sx_i col
      gy_j  = Y_j^T [Y_j|1]              -> |y_j|^2 diagonal, Sy_j col
  - Act evacuates PSUM to SBUF f32; DVE extracts the diagonals with masked
    scalar_tensor_tensor accumulations (identity / triple-shifted-diag masks).
  - Final stage (tiny [128, 4] fp32 tiles): centroid corrections, 3x3 C^T C
    eigenvalues via the closed-form trigonometric method, Kabsch det sign,
    RMSD.
"""

import numpy as np

import concourse.bass as bass
import concourse.mybir as mybir
from concourse.tile import TileContext, ScopedClock

F32 = mybir.dt.float32
FP8 = mybir.dt.float8e4
OP = mybir.AluOpType
AF = mybir.ActivationFunctionType
MM = mybir.MatmulPerfMode

N_CORES = 8
ROWS = 128      # rows per tile == SBUF partitions == matmul stationary cols
CHUNK = 256     # atoms per DoubleRow matmul
BLK = 258       # fp8 cols per component block: (128 rows + 1 ones) * 2 ktiles
CW = 3 * BLK    # per-chunk width (3 components)
GW = 390        # gx/gy psum width: 3 blocks of 129 at 130 spacing
JW = 387        # j-gram width: 3 blocks of 129
CPW = 3 * JW + 2 * GW  # per-class copy width (1941)


# ---------------------------------------------------------------------------
# TileContext tail patch: this walrus build accepts at most ONE sync-wait
# command per instruction and no sem-eq waits, so the stock drain + EVSEM
# butterfly fails codegen. Emit a ge-wait-only tail instead.
# ---------------------------------------------------------------------------
def _patched_drain_and_barrier(self, tick_clock, wait_clock):
    nc = self.nc
    dummy = nc.gpsimd.nop()
    wait_clock.add_sem_waits(dummy.ins, ScopedClock({None: tick_clock.global_clock}))
    waits = list(dummy.ins.sync_info.on_wait) if dummy.ins.sync_info else []
    if dummy.ins.sync_info:
        dummy.ins.sync_info = mybir.SyncInfo(on_wait=[], on_update=[])

    bsem = nc.alloc_semaphore(f"tail_bsem_{nc.next_id()}")
    dsem = nc.alloc_semaphore(f"tail_dsem_{nc.next_id()}")
    n_eng = 0
    for eng in nc.engines.values():
        eng.drain()
        eng.sem_inc(bsem, 1)
        n_eng += 1
    nc.gpsimd.wait_ge(bsem, n_eng)
    for w in waits:
        n = nc.gpsimd.nop()
        n.ins.sync_info = mybir.SyncInfo(on_wait=[w], on_update=[])
    nc.gpsimd.sem_inc(dsem, 1)
    for eng in nc.engines.values():
        if eng is not nc.gpsimd:
            eng.wait_ge(dsem, 1)

    popped = nc._tile_sem_poison_stack.pop()
    assert popped is self._sem_poison
    nc.clear_and_free_semaphores(list(self.sems.allocated().values()))
    nc.gpsimd.sem_clear(bsem)
    nc.gpsimd.sem_clear(dsem)


def install_tile_patch():
    TileContext._drain_and_barrier = _patched_drain_and_barrier


# ---------------------------------------------------------------------------
# BIR post-pass: this walrus build accepts at most one sync-wait command per
# instruction (none on Drain). Tile's sem-assigner can attach several, so
# split extras onto same-engine NoOps inserted just before the instruction.
# ---------------------------------------------------------------------------
_orig_to_json_bytes = bass.Bass.to_json_bytes


def _split_multiwait_json(self) -> bytes:
    import json

    raw = _orig_to_json_bytes(self)
    m = json.loads(raw)
    ctr = 0
    changed = False
    for f in m.get("functions", []):
        for blk in f.get("blocks", []):
            insts = blk.get("instructions", [])
            out = []
            for inst in insts:
                si = inst.get("sync_info")
                ow = (si or {}).get("on_wait") or []
                opc = str(inst.get("opcode", inst.get("type", "")))
                limit = 0 if opc == "Drain" else 1
                if len(ow) > limit:
                    keep = ow[len(ow) - limit :] if limit else []
                    moved = ow[: len(ow) - limit] if limit else ow
                    for w in moved:
                        ctr += 1
                        out.append(
                            {
                                "debug": inst.get("debug", 0),
                                "engine": inst["engine"],
                                "ins": [],
                                "name": f"WS-{ctr}-{inst['name']}",
                                "opcode": "NoOp",
                                "outs": [],
                                "sync_info": {"on_update": [], "on_wait": [w]},
                            }
                        )
                    si["on_wait"] = keep
                    changed = True
                out.append(inst)
            blk["instructions"] = out
    if not changed:
        return raw
    return json.dumps(m).encode()


bass.Bass.to_json_bytes = _split_multiwait_json


# ---------------------------------------------------------------------------
# Final math emitter: batched wide fp32 ops, split across DVE/Act/Pool.
# ---------------------------------------------------------------------------
class _FM:
    def __init__(self, nc, pool, K):
        self.nc = nc
        self.pool = pool
        self.K = K
        self.n = 0
        self._consts = {}

    def const_col(self, val):
        val = float(val)
        if val in self._consts:
            return self._consts[val]
        i = len(self._consts)
        t = self.pool.tile([ROWS, 1], F32, tag=f"fmc{i}", name=f"fmc{i}")
        self.nc.vector.memset(t[:], val)
        self._consts[val] = t[:]
        return t[:]

    def t(self, w=None):
        self.n += 1
        w = w or self.K
        return self.pool.tile([ROWS, w], F32, tag=f"fm{self.n}", name=f"fm{self.n}")

    def tt(self, a, b, op, w=None, eng=None):
        o = self.t(w)
        (eng or self.nc.vector).tensor_tensor(o[:], a, b, op)
        return o[:]

    def mul(self, a, b, w=None, eng=None):
        o = self.t(w)
        eng = eng or self.nc.vector
        if eng is self.nc.gpsimd:
            eng.tensor_mul(o[:], a, b)
        else:
            eng.tensor_tensor(o[:], a, b, OP.mult)
        return o[:]

    def add(self, a, b, w=None, eng=None):
        o = self.t(w)
        eng = eng or self.nc.vector
        if eng is self.nc.gpsimd:
            eng.tensor_add(o[:], a, b)
        else:
            eng.tensor_tensor(o[:], a, b, OP.add)
        return o[:]

    def sub(self, a, b, w=None, eng=None):
        o = self.t(w)
        eng = eng or self.nc.vector
        if eng is self.nc.gpsimd:
            eng.tensor_sub(o[:], a, b)
        else:
            eng.tensor_tensor(o[:], a, b, OP.subtract)
        return o[:]

    def ts(self, a, s, op, w=None):
        o = self.t(w)
        self.nc.vector.tensor_scalar(o[:], a, float(s), None, op)
        return o[:]

    def ts2(self, a, s1, s2, op0, op1):
        o = self.t()
        self.nc.vector.tensor_scalar(o[:], a, float(s1), float(s2), op0, op1)
        return o[:]

    def stt(self, a, s, b, op0, op1, w=None):
        """(a op0 s) op1 b"""
        o = self.t(w)
        self.nc.vector.scalar_tensor_tensor(o[:], a, float(s), b, op0, op1)
        return o[:]

    def act(self, a, func, bias=0.0, scale=1.0, w=None):
        o = self.t(w)
        if isinstance(bias, float) and bias not in (0.0, 1.0) and func != AF.Copy:
            bias = self.const_col(bias)
        self.nc.scalar.activation(o[:], a, func, bias=bias, scale=scale)
        return o[:]

    def recip(self, a):
        o = self.t()
        self.nc.vector.reciprocal(o[:], a)
        return o[:]


def _emit_final_math(nc, fm, st_rxy, st_g, sxp, syp, n_ap, out_ap):
    """st_rxy: [128, 9K] raw moments, col (3i+j)K+k; st_g: [128, K] (Gx+Gy);
    sxp/syp: [128, 3K] packed raw sums, col iK+k; n_ap: [128, K]."""
    K = fm.K
    pool_eng = nc.gpsimd
    rn = fm.recip(n_ap)

    # C = st_rxy - (Sx_i Sy_j) rn : three wide [128, 9K] ops
    sxA = sxp.rearrange("p (i k) -> p i k", i=3).unsqueeze(2).broadcast_to([ROWS, 3, 3, K])
    syB = syp.rearrange("p (j k) -> p j k", j=3).unsqueeze(1).broadcast_to([ROWS, 3, 3, K])
    t1 = fm.t(9 * K)
    nc.vector.tensor_tensor(t1[:].rearrange("p (i j k) -> p i j k", i=3, j=3), sxA, syB, OP.mult)
    rn9 = rn.unsqueeze(1).broadcast_to([ROWS, 9, K])
    t2 = fm.t(9 * K)
    nc.vector.tensor_tensor(t2[:].rearrange("p (g k) -> p g k", g=9), t1[:].rearrange("p (g k) -> p g k", g=9), rn9, OP.mult)
    Ct = fm.sub(st_rxy, t2[:], w=9 * K)
    Cr = Ct.rearrange("p (i j k) -> p i j k", i=3, j=3)

    def C(i, j):
        return Ct[:, (3 * i + j) * K : (3 * i + j + 1) * K]

    # gxy = (Gx + Gy raw, pre-merged in st_g) - (|Sx|^2 + |Sy|^2) rn  (Pool)
    sqx = fm.mul(sxp, sxp, w=3 * K, eng=pool_eng)
    sqy = fm.mul(syp, syp, w=3 * K, eng=pool_eng)
    sq1 = fm.add(sqx, sqy, w=3 * K, eng=pool_eng)
    sq2 = fm.add(sq1[:, 0:K], sq1[:, K : 2 * K], eng=pool_eng)
    sq3 = fm.add(sq2, sq1[:, 2 * K : 3 * K], eng=pool_eng)
    sqrn = fm.mul(sq3, rn, eng=pool_eng)
    gxy = fm.sub(st_g, sqrn, eng=pool_eng)

    # K = C^T C packed into kkp [128, 6K]: order [00, 11, 22, 01, 02, 12]
    kkp = fm.t(6 * K)
    for idx, (a, b) in enumerate([(0, 0), (1, 1), (2, 2), (0, 1), (0, 2), (1, 2)]):
        m = fm.t(3 * K)
        nc.vector.tensor_tensor(
            m[:].rearrange("p (i k) -> p i k", i=3), Cr[:, :, a, :], Cr[:, :, b, :], OP.mult
        )
        f1 = fm.add(m[:, 0:K], m[:, K : 2 * K])
        nc.vector.tensor_tensor(
            kkp[:, idx * K : (idx + 1) * K], f1, m[:, 2 * K : 3 * K], OP.add
        )
    kdiag = kkp[:, 0 : 3 * K]
    koff = kkp[:, 3 * K : 6 * K]
    k01 = kkp[:, 3 * K : 4 * K]
    k02 = kkp[:, 4 * K : 5 * K]
    k12 = kkp[:, 5 * K : 6 * K]

    # det(C)  (Pool branch; only feeds the Kabsch sign)
    m0 = fm.sub(fm.mul(C(1, 1), C(2, 2), eng=pool_eng), fm.mul(C(1, 2), C(2, 1), eng=pool_eng), eng=pool_eng)
    m1 = fm.sub(fm.mul(C(1, 0), C(2, 2), eng=pool_eng), fm.mul(C(1, 2), C(2, 0), eng=pool_eng), eng=pool_eng)
    m2 = fm.sub(fm.mul(C(1, 0), C(2, 1), eng=pool_eng), fm.mul(C(1, 1), C(2, 0), eng=pool_eng), eng=pool_eng)
    d0 = fm.mul(C(0, 0), m0, eng=pool_eng)
    d1 = fm.mul(C(0, 1), m1, eng=pool_eng)
    d2 = fm.mul(C(0, 2), m2, eng=pool_eng)
    detC = fm.add(fm.sub(d0, d1, eng=pool_eng), d2, eng=pool_eng)
    # Kabsch sign d = 1 - 2*(detC < 0)  (stay on Pool-adjacent DVE ops)
    neg = fm.ts(detC, 0.0, OP.is_lt)
    dsg = fm.ts2(neg, -2.0, 1.0, OP.mult, OP.add)

    # q = tr(K)/3 ; kd = diag - q (one wide op); p2 = |kd|^2 + 2 |koff|^2
    trk = fm.add(fm.add(kkp[:, 0:K], kkp[:, K : 2 * K]), kkp[:, 2 * K : 3 * K])
    q = fm.ts(trk, 1.0 / 3.0, OP.mult)
    kdp = fm.t(3 * K)
    q3 = q.unsqueeze(1).broadcast_to([ROWS, 3, K])
    nc.vector.tensor_tensor(
        kdp[:].rearrange("p (i k) -> p i k", i=3),
        kdiag.rearrange("p (i k) -> p i k", i=3), q3, OP.subtract,
    )
    sq_all = fm.t(6 * K)
    nc.vector.tensor_tensor(sq_all[:], kkp[:], kkp[:], OP.mult)  # squares of all 6
    # p2 = sum over diag squares... need kd squares, not kk squares; redo:
    kdsq = fm.mul(kdp[:], kdp[:], w=3 * K)
    p2a = fm.add(fm.add(kdsq[:, 0:K], kdsq[:, K : 2 * K]), kdsq[:, 2 * K : 3 * K])
    xsq = fm.add(
        fm.add(sq_all[:, 3 * K : 4 * K], sq_all[:, 4 * K : 5 * K]),
        sq_all[:, 5 * K : 6 * K],
    )
    p2 = fm.stt(xsq, 2.0, p2a, OP.mult, OP.add)  # p2a + 2*xsq
    p2c = fm.ts2(p2, 1.0 / 6.0, 1e-30, OP.mult, OP.max)
    p = fm.act(p2c, AF.Sqrt)

    # det(K - qI) (symmetric)
    e0 = fm.mul(kdp[:, 0:K], fm.sub(fm.mul(kdp[:, K : 2 * K], kdp[:, 2 * K : 3 * K]), sq_all[:, 5 * K : 6 * K]))
    e1 = fm.mul(k01, fm.sub(fm.mul(k01, kdp[:, 2 * K : 3 * K]), fm.mul(k12, k02)))
    e2 = fm.mul(k02, fm.sub(fm.mul(k01, k12), fm.mul(kdp[:, K : 2 * K], k02)))
    detKq = fm.add(fm.sub(e0, e1), e2)

    # r = 0.5 det(K-qI) / p^3, clamped to [-1, 1]
    rp = fm.recip(p)
    r = fm.mul(fm.mul(fm.ts(detKq, 0.5, OP.mult), rp), fm.mul(rp, rp))
    r = fm.ts2(r, 1.0, -1.0, OP.min, OP.max)

    # acos via |r| fold:
    #   A = 2 atan(sqrt((1-|r|)/(1+|r|))) = acos(|r|)
    #   acos(r) = A + (r<0) * (pi - 2A) ; phi = acos(r)/3
    rabs = fm.stt(r, -1.0, r, OP.mult, OP.max)  # |r| = max(-r, r)
    onemr = fm.ts2(rabs, -1.0, 1.0, OP.mult, OP.add)  # 1 - |r|
    onepr = fm.ts(rabs, 1.0, OP.add)
    u = fm.mul(onemr, fm.recip(onepr))
    su = fm.act(u, AF.Sqrt)
    at = fm.act(su, AF.Arctan)  # Act->Act, single round trip
    A = fm.ts(at, 2.0, OP.mult)
    rneg = fm.ts(r, 0.0, OP.is_lt)
    corr = fm.ts2(A, -2.0, float(np.pi), OP.mult, OP.add)  # pi - 2A
    acr = fm.add(A, fm.mul(rneg, corr))
    # cos(phi) = sin(phi/1 + pi/2) ; cos(phi + 2pi/3) = -sin(5pi/6 - phi)
    # pack both sin args into one [128, 2K] tile -> single Act Sin
    sargs = fm.t(2 * K)
    nc.vector.tensor_scalar(sargs[:, 0:K], acr, 1.0 / 3.0, float(np.pi / 2), OP.mult, OP.add)
    nc.vector.tensor_scalar(sargs[:, K : 2 * K], acr, -1.0 / 3.0, float(5 * np.pi / 6), OP.mult, OP.add)
    sc = fm.act(sargs[:], AF.Sin, w=2 * K)

    # eigenvalues packed: l = [l1 | l2 | l3], one clamp + one sqrt
    p2x = fm.ts(p, 2.0, OP.mult)
    lp = fm.t(3 * K)
    l1 = fm.add(q, fm.mul(p2x, sc[:, 0:K]))
    l3 = fm.sub(q, fm.mul(p2x, sc[:, K : 2 * K]))
    nc.vector.tensor_copy(lp[:, 0:K], l1)
    nc.vector.tensor_copy(lp[:, 2 * K : 3 * K], l3)
    nc.vector.tensor_tensor(
        lp[:, K : 2 * K], fm.stt(q, 3.0, l1, OP.mult, OP.subtract), l3, OP.subtract
    )
    lc = fm.ts(lp[:], 0.0, OP.max, w=3 * K)
    sv = fm.act(lc, AF.Sqrt, w=3 * K)

    tr = fm.add(fm.add(sv[:, 0:K], sv[:, K : 2 * K]), fm.mul(dsg, sv[:, 2 * K : 3 * K]))

    # msd = (gxy - 2 tr) rn ; rmsd = sqrt(max(msd, 0))
    diff = fm.stt(tr, -2.0, gxy, OP.mult, OP.add)
    msd = fm.mul(diff, rn)
    rmsd = fm.act(fm.ts(msd, 0.0, OP.max), AF.Sqrt)
    nc.vector.tensor_copy(out_ap, rmsd)


# ---------------------------------------------------------------------------
# Program builder
# ---------------------------------------------------------------------------
def build_program(caps, nmax, cfg=None):
    """caps: per-class atom capacities (multiples of 256). Returns nc."""
    cfg = cfg or {}
    K = len(caps)
    chunks = [c // CHUNK for c in caps]
    assert all(c * CHUNK == cap for c, cap in zip(chunks, caps))
    offs = np.cumsum([0] + [c * CW for c in chunks])
    TOTW = int(offs[-1])

    install_tile_patch()
    nc = bass.Bass()
    x_d = nc.dram_tensor("x", [ROWS, TOTW], FP8, kind="ExternalInput")
    y_d = nc.dram_tensor("y", [ROWS, TOTW], FP8, kind="ExternalInput")
    # consts packed: [meta (K) | imask (128) | gmask (780)]
    CONW = K + ROWS + 2 * GW
    consts_d = nc.dram_tensor("consts", [ROWS, CONW], F32, kind="ExternalInput")
    out_d = nc.dram_tensor("out", [ROWS, K], F32, kind="ExternalOutput")

    with TileContext(nc) as tc:
        with (
            tc.tile_pool(name="const", bufs=1) as constp,
            tc.tile_pool(name="data", bufs=1) as datap,
            tc.tile_pool(name="stats", bufs=1) as statp,
            tc.tile_pool(name="cps", bufs=2) as cpsp,
            tc.psum_pool(name="pj", bufs=2) as pjp,
            tc.psum_pool(name="pg", bufs=1) as pgp,
        ):
            consts_t = constp.tile([ROWS, CONW], F32)
            nc.sync.dma_start(out=consts_t[:], in_=consts_d[:])
            meta_t = consts_t[:, 0:K]
            imask_t = consts_t[:, K : K + ROWS]
            gmask_t = consts_t[:, K + ROWS : CONW]

            # Smallest class first: its matmuls finish earliest, so the Act
            # copy queue starts draining as soon as possible.
            korder = list(range(K))
            x_t = datap.tile([ROWS, TOTW], FP8, name="x_t", tag="x_t")
            y_t = datap.tile([ROWS, TOTW], FP8, name="y_t", tag="y_t")
            for k in korder:
                a, b = int(offs[k]), int(offs[k + 1])
                nc.sync.dma_start(out=x_t[:, a:b], in_=x_d[:, a:b])
                nc.sync.dma_start(out=y_t[:, a:b], in_=y_d[:, a:b])

            st_rxy = statp.tile([ROWS, 9 * K], F32)
            st_g = statp.tile([ROWS, K], F32)
            sxp = statp.tile([ROWS, 3 * K], F32, name="sxp", tag="sxp")
            syp = statp.tile([ROWS, 3 * K], F32, name="syp", tag="syp")
            # Rotating scratch tiles: extraction accums are independent, a
            # single scratch would serialize them through WAW hazards.
            scrs = [
                statp.tile([ROWS, 2 * GW], F32, name=f"scr{i}", tag=f"scr{i}")
                for i in range(4)
            ]
            _scr_n = [0]

            def scr_next():
                t = scrs[_scr_n[0] % len(scrs)]
                _scr_n[0] += 1
                return t

            phase = cfg.get("phase", "full")  # dma|mm|copies|extract|full

            for k in korder:
                if phase == "dma":
                    break
                base_k = int(offs[k])
                nck = chunks[k]
                oj = [
                    pjp.tile([ROWS, JW], F32, name=f"oj{j}", tag=f"oj{j}")
                    for j in range(3)
                ]
                ogx = pgp.tile([ROWS, GW], F32, name="ogx", tag="ogx")
                ogy = pgp.tile([ROWS, GW], F32, name="ogy", tag="ogy")
                # Multiple accumulation groups share the ogx/ogy banks and
                # start=True zeroes a whole bank, so only the very first
                # matmul of each bank carries start=True.

                # Group-major matmul order: each gram's accumulation group
                # finishes as early as possible so its copy/extraction can
                # start while later groups still run on PE.
                def rhs_all(cb):
                    return x_t[:, cb : cb + CW].rearrange(
                        "p (b c2 two) -> p two b c2", two=2, b=3
                    )[:, :, :, ::-1]

                def rhs_blk(t, cb, i):
                    sl = slice(cb + BLK * i, cb + BLK * (i + 1))
                    return t[:, sl].rearrange("p (c2 two) -> p two c2", two=2)[:, :, ::-1]

                cj = [
                    cpsp.tile([ROWS, JW], F32, name=f"cj{j}", tag=f"cj{j}")
                    for j in range(3)
                ]
                cg = cpsp.tile([ROWS, 2 * GW], F32, name="cg", tag="cg")
                do_ce = phase not in ("mm",)

                for i in range(3):
                    for c in range(nck):
                        cb = base_k + c * CW
                        nc.tensor.matmul(
                            ogx[:, 130 * i : 130 * i + 129],
                            x_t[:, cb + BLK * i + 2 : cb + BLK * (i + 1)],
                            rhs_blk(x_t, cb, i),
                            start=(c == 0 and i == 0), stop=(c == nck - 1),
                            perf_mode=MM.DoubleRowSwInterleave,
                            skip_group_check=True,
                        )
                for j in range(3):
                    for c in range(nck):
                        cb = base_k + c * CW
                        nc.tensor.matmul(
                            ogy[:, 130 * j : 130 * j + 129],
                            y_t[:, cb + BLK * j + 2 : cb + BLK * (j + 1)],
                            rhs_blk(y_t, cb, j),
                            start=(c == 0 and j == 0), stop=(c == nck - 1),
                            perf_mode=MM.DoubleRowSwInterleave,
                            skip_group_check=True,
                        )
                if do_ce:
                    nc.scalar.copy(cg[:, 0:GW], ogx[:])
                    nc.scalar.copy(cg[:, GW : 2 * GW], ogy[:])
                for j in range(3):
                    for c in range(nck):
                        cb = base_k + c * CW
                        nc.tensor.matmul(
                            oj[j][:],
                            y_t[:, cb + BLK * j + 2 : cb + BLK * (j + 1)],
                            rhs_all(cb),
                            start=(c == 0), stop=(c == nck - 1),
                            perf_mode=MM.DoubleRowSwInterleave,
                        )
                    if do_ce:
                        nc.scalar.copy(cj[j][:], oj[j][:])

                if phase in ("mm", "copies"):
                    continue
                # DVE: masked diagonal extraction
                # st_rxy layout: col (3i+j)*K + k ; st_g: Gx+Gy at col k
                nc.vector.scalar_tensor_tensor(
                    scr_next()[:], cg[:], 1.0, gmask_t, OP.mult, OP.mult,
                    accum_out=st_g[:, k : k + 1],
                )
                # Raw sums from the ones-columns (cols 128::130 of each bank):
                # sxp/syp [128, 3K], col i*K + k.
                nc.vector.tensor_copy(sxp[:, k : 2 * K + k + 1 : K], cg[:, 128:GW:130])
                nc.vector.tensor_copy(syp[:, k : 2 * K + k + 1 : K], cg[:, GW + 128 : 2 * GW : 130])
                for j in range(3):
                    for i in range(3):
                        col = (3 * i + j) * K + k
                        nc.vector.scalar_tensor_tensor(
                            scr_next()[:, 0:ROWS],
                            cj[j][:, 129 * i : 129 * i + ROWS],
                            1.0, imask_t, OP.mult, OP.mult,
                            accum_out=st_rxy[:, col : col + 1],
                        )

            out_t = statp.tile([ROWS, K], F32)
            if phase == "full":
                fm = _FM(nc, statp, K)
                _emit_final_math(
                    nc, fm, st_rxy[:], st_g[:], sxp[:], syp[:], meta_t, out_t[:]
                )
            else:
                nc.vector.memset(out_t[:], 0.0)
            nc.sync.dma_start(out=out_d[:], in_=out_t[:])

            if cfg.get("debug"):
                dbg_rxy = nc.dram_tensor("dbg_rxy", [ROWS, 9 * K], F32, kind="ExternalOutput")
                dbg_g = nc.dram_tensor("dbg_g", [ROWS, K], F32, kind="ExternalOutput")
                dbg_sx = nc.dram_tensor("dbg_sx", [ROWS, 3 * K], F32, kind="ExternalOutput")
                dbg_sy = nc.dram_tensor("dbg_sy", [ROWS, 3 * K], F32, kind="ExternalOutput")
                nc.sync.dma_start(out=dbg_rxy[:], in_=st_rxy[:])
                nc.sync.dma_start(out=dbg_g[:], in_=st_g[:])
                nc.sync.dma_start(out=dbg_sx[:], in_=sxp[:])
                nc.sync.dma_start(out=dbg_sy[:], in_=syp[:])

    return nc


# ---------------------------------------------------------------------------
# Host side
# ---------------------------------------------------------------------------
def plan_shards(num_atoms, n_classes=4, cap_round=CHUNK):
    B = num_atoms.shape[0]
    assert B % (N_CORES * ROWS) == 0
    n_classes_total = B // (N_CORES * ROWS)
    assert n_classes == n_classes_total
    order = np.argsort(num_atoms, kind="stable")
    na_sorted = num_atoms[order]
    rows_per_class = N_CORES * ROWS
    caps = []
    for k in range(n_classes):
        mx = int(na_sorted[(k + 1) * rows_per_class - 1])
        cap = ((mx + cap_round - 1) // cap_round) * cap_round
        caps.append(cap)
    return order, caps


def shard_inputs(coords_input, coords_target, num_atoms, order, caps, nmax):
    import ml_dtypes

    K = len(caps)
    rows_per_class = N_CORES * ROWS
    chunks = [c // CHUNK for c in caps]
    TOTW = sum(c * CW for c in chunks)

    imask = np.eye(ROWS, dtype=np.float32)
    gmask = np.zeros((ROWS, 2 * GW), dtype=np.float32)
    r = np.arange(ROWS)
    for i in range(3):
        gmask[r, 130 * i + r] = 1.0
        gmask[r, GW + 130 * i + r] = 1.0

    in_maps = []
    core_row_idx = []
    for c in range(N_CORES):
        idx = np.concatenate(
            [
                order[k * rows_per_class + c * ROWS : k * rows_per_class + (c + 1) * ROWS]
                for k in range(K)
            ]
        )
        core_row_idx.append(idx)
        na_c = num_atoms[idx].astype(np.int64)  # [K*ROWS]

        bufs = {"x": np.zeros((ROWS, TOTW), dtype=ml_dtypes.float8_e4m3fn),
                "y": np.zeros((ROWS, TOTW), dtype=ml_dtypes.float8_e4m3fn)}
        off = 0
        for k in range(K):
            rows_k = idx[k * ROWS : (k + 1) * ROWS]
            na_k = na_c[k * ROWS : (k + 1) * ROWS]
            cap = caps[k]
            nck = chunks[k]
            amask = (np.arange(cap)[None, :] < na_k[:, None])
            for nm, coords in (("x", coords_input), ("y", coords_target)):
                v = coords[rows_k, : 3 * cap].reshape(ROWS, cap, 3)
                v = np.where(amask[:, :, None], v, 0.0).astype(np.float32)
                # [r, ch, t, p, i] -> [p, ch, i, r, t]
                v = v.reshape(ROWS, nck, 2, 128, 3).transpose(3, 1, 4, 0, 2)
                blk = np.zeros((128, nck, 3, 129, 2), dtype=ml_dtypes.float8_e4m3fn)
                blk[:, :, :, 1:, :] = v[:, :, :, ::-1, :].astype(
                    ml_dtypes.float8_e4m3fn
                )
                blk[:, :, :, 0, :] = 1.0
                w = nck * CW
                bufs[nm][:, off : off + w] = blk.reshape(128, w)
            off += nck * CW

        meta = np.ascontiguousarray(na_c.astype(np.float32).reshape(K, ROWS).T)
        consts = np.concatenate([meta, imask, gmask], axis=1)
        in_maps.append({"x": bufs["x"], "y": bufs["y"], "consts": consts})
    return in_maps, core_row_idx


def unshard_outputs(results, core_row_idx, B):
    out = np.empty(B, dtype=np.float32)
    for c in range(N_CORES):
        o = results[c]["out"]  # [ROWS, K]
        idx = core_row_idx[c]
        out[idx] = o.T.reshape(-1)
    return out


# ---------------------------------------------------------------------------
# Entry point: full inputs in, full output out. Shards across 8 NeuronCores.
# ---------------------------------------------------------------------------
_PROG_CACHE = {}


def _get_program(caps, nmax):
    key = (tuple(caps), nmax)
    if key not in _PROG_CACHE:
        _PROG_CACHE[key] = build_program(list(caps), nmax)
    return _PROG_CACHE[key]


def kernel(coords_input, coords_target, num_atoms):
    from concourse.bass_utils import run_bass_kernel_spmd

    x = np.ascontiguousarray(np.asarray(coords_input, dtype=np.float32))
    y = np.ascontiguousarray(np.asarray(coords_target, dtype=np.float32))
    na = np.asarray(num_atoms)
    na_i = na.astype(np.int64)
    B, ncols = x.shape
    nmax = ncols // 3
    K = B // (N_CORES * ROWS)
    assert B == N_CORES * ROWS * K, f"unsupported batch {B}"

    order, caps = plan_shards(na_i, n_classes=K)
    in_maps, core_row_idx = shard_inputs(x, y, na_i, order, caps, nmax)
    nc = _get_program(caps, nmax)
    res = run_bass_kernel_spmd(nc, in_maps, core_ids=list(range(N_CORES)))
    out = unshard_outputs(res.results, core_row_idx, B)
    return out.astype(np.float32)


# revision 48
# speedup vs baseline: 1.2060x; 1.0254x over previous
"""Bass/Trainium2 kernel for batched masked-Kabsch RMSD (nn_Coords2RMSD).

Strategy (per NeuronCore, SPMD across 8 cores):
  - Host sorts batch rows by num_atoms into 4 size classes (quartiles); core c
    takes one 128-row tile from each class, capped at the class max atom count
    rounded to 256. Padded atoms are zeroed on the host so they drop out of
    every statistic.
  - Data is uploaded fp8-e4m3 in a transposed, pair-interleaved layout
    ([atom-in-chunk partitions] x [chunk, component, (row, ktile) pairs],
    column-reversed per the DoubleRowSwInterleave weight format, with a ones
    column per component block).
  - The TensorEngine computes, per class, Gram blocks accumulated in PSUM via
    fp8 DoubleRowSwInterleave matmuls (256 atoms per instruction):
      out_j = Y_j^T [X_0|1|X_1|1|X_2|1]  -> R_ij on block diagonals, Sy_j cols
      gx_i  = X_i^T [X_i|1]              -> |x_i|^2 diagonal, Sx_i col
      gy_j  = Y_j^T [Y_j|1]              -> |y_j|^2 diagonal, Sy_j col
  - Act evacuates PSUM to SBUF f32; DVE extracts the diagonals with masked
    scalar_tensor_tensor accumulations (identity / triple-shifted-diag masks).
  - Final stage (tiny [128, 4] fp32 tiles): centroid corrections, 3x3 C^T C
    eigenvalues via the closed-form trigonometric method, Kabsch det sign,
    RMSD.
"""

import numpy as np

import concourse.bass as bass
import concourse.mybir as mybir
from concourse.tile import TileContext, ScopedClock

F32 = mybir.dt.float32
BF16 = mybir.dt.bfloat16
FP8 = mybir.dt.float8e4
OP = mybir.AluOpType
AF = mybir.ActivationFunctionType
MM = mybir.MatmulPerfMode

N_CORES = 8
ROWS = 128      # rows per tile == SBUF partitions == matmul stationary cols
CHUNK = 256     # atoms per DoubleRow matmul
BLK = 258       # fp8 cols per component block: (128 rows + 1 ones) * 2 ktiles
CW = 3 * BLK    # per-chunk width (3 components)
GW = 390        # gx/gy psum width: 3 blocks of 129 at 130 spacing
JW = 387        # j-gram width: 3 blocks of 129
CPW = 3 * JW + 2 * GW  # per-class copy width (1941)


# ---------------------------------------------------------------------------
# TileContext tail patch: this walrus build accepts at most ONE sync-wait
# command per instruction and no sem-eq waits, so the stock drain + EVSEM
# butterfly fails codegen. Emit a ge-wait-only tail instead.
# ---------------------------------------------------------------------------
def _patched_drain_and_barrier(self, tick_clock, wait_clock):
    nc = self.nc
    dummy = nc.gpsimd.nop()
    wait_clock.add_sem_waits(dummy.ins, ScopedClock({None: tick_clock.global_clock}))
    waits = list(dummy.ins.sync_info.on_wait) if dummy.ins.sync_info else []
    if dummy.ins.sync_info:
        dummy.ins.sync_info = mybir.SyncInfo(on_wait=[], on_update=[])

    bsem = nc.alloc_semaphore(f"tail_bsem_{nc.next_id()}")
    dsem = nc.alloc_semaphore(f"tail_dsem_{nc.next_id()}")
    n_eng = 0
    for eng in nc.engines.values():
        eng.drain()
        eng.sem_inc(bsem, 1)
        n_eng += 1
    nc.gpsimd.wait_ge(bsem, n_eng)
    for w in waits:
        n = nc.gpsimd.nop()
        n.ins.sync_info = mybir.SyncInfo(on_wait=[w], on_update=[])
    nc.gpsimd.sem_inc(dsem, 1)
    for eng in nc.engines.values():
        if eng is not nc.gpsimd:
            eng.wait_ge(dsem, 1)

    popped = nc._tile_sem_poison_stack.pop()
    assert popped is self._sem_poison
    nc.clear_and_free_semaphores(list(self.sems.allocated().values()))
    nc.gpsimd.sem_clear(bsem)
    nc.gpsimd.sem_clear(dsem)


def install_tile_patch():
    TileContext._drain_and_barrier = _patched_drain_and_barrier


# ---------------------------------------------------------------------------
# BIR post-pass: this walrus build accepts at most one sync-wait command per
# instruction (none on Drain). Tile's sem-assigner can attach several, so
# split extras onto same-engine NoOps inserted just before the instruction.
# ---------------------------------------------------------------------------
_orig_to_json_bytes = bass.Bass.to_json_bytes


def _split_multiwait_json(self) -> bytes:
    import json

    raw = _orig_to_json_bytes(self)
    m = json.loads(raw)
    ctr = 0
    changed = False
    for f in m.get("functions", []):
        for blk in f.get("blocks", []):
            insts = blk.get("instructions", [])
            out = []
            for inst in insts:
                si = inst.get("sync_info")
                ow = (si or {}).get("on_wait") or []
                opc = str(inst.get("opcode", inst.get("type", "")))
                limit = 0 if opc == "Drain" else 1
                if len(ow) > limit:
                    keep = ow[len(ow) - limit :] if limit else []
                    moved = ow[: len(ow) - limit] if limit else ow
                    for w in moved:
                        ctr += 1
                        out.append(
                            {
                                "debug": inst.get("debug", 0),
                                "engine": inst["engine"],
                                "ins": [],
                                "name": f"WS-{ctr}-{inst['name']}",
                                "opcode": "NoOp",
                                "outs": [],
                                "sync_info": {"on_update": [], "on_wait": [w]},
                            }
                        )
                    si["on_wait"] = keep
                    changed = True
                out.append(inst)
            blk["instructions"] = out
    if not changed:
        return raw
    return json.dumps(m).encode()


bass.Bass.to_json_bytes = _split_multiwait_json


# ---------------------------------------------------------------------------
# Final math emitter: batched wide fp32 ops, split across DVE/Act/Pool.
# ---------------------------------------------------------------------------
class _FM:
    def __init__(self, nc, pool, K):
        self.nc = nc
        self.pool = pool
        self.K = K
        self.n = 0
        self._consts = {}

    def const_col(self, val):
        val = float(val)
        if val in self._consts:
            return self._consts[val]
        i = len(self._consts)
        t = self.pool.tile([ROWS, 1], F32, tag=f"fmc{i}", name=f"fmc{i}")
        self.nc.vector.memset(t[:], val)
        self._consts[val] = t[:]
        return t[:]

    def t(self, w=None):
        self.n += 1
        w = w or self.K
        return self.pool.tile([ROWS, w], F32, tag=f"fm{self.n}", name=f"fm{self.n}")

    def tt(self, a, b, op, w=None, eng=None):
        o = self.t(w)
        (eng or self.nc.vector).tensor_tensor(o[:], a, b, op)
        return o[:]

    def mul(self, a, b, w=None, eng=None):
        o = self.t(w)
        eng = eng or self.nc.vector
        if eng is self.nc.gpsimd:
            eng.tensor_mul(o[:], a, b)
        else:
            eng.tensor_tensor(o[:], a, b, OP.mult)
        return o[:]

    def add(self, a, b, w=None, eng=None):
        o = self.t(w)
        eng = eng or self.nc.vector
        if eng is self.nc.gpsimd:
            eng.tensor_add(o[:], a, b)
        else:
            eng.tensor_tensor(o[:], a, b, OP.add)
        return o[:]

    def sub(self, a, b, w=None, eng=None):
        o = self.t(w)
        eng = eng or self.nc.vector
        if eng is self.nc.gpsimd:
            eng.tensor_sub(o[:], a, b)
        else:
            eng.tensor_tensor(o[:], a, b, OP.subtract)
        return o[:]

    def ts(self, a, s, op, w=None):
        o = self.t(w)
        self.nc.vector.tensor_scalar(o[:], a, float(s), None, op)
        return o[:]

    def ts2(self, a, s1, s2, op0, op1):
        o = self.t()
        self.nc.vector.tensor_scalar(o[:], a, float(s1), float(s2), op0, op1)
        return o[:]

    def stt(self, a, s, b, op0, op1, w=None):
        """(a op0 s) op1 b"""
        o = self.t(w)
        self.nc.vector.scalar_tensor_tensor(o[:], a, float(s), b, op0, op1)
        return o[:]

    def act(self, a, func, bias=0.0, scale=1.0, w=None):
        o = self.t(w)
        if isinstance(bias, float) and bias not in (0.0, 1.0) and func != AF.Copy:
            bias = self.const_col(bias)
        self.nc.scalar.activation(o[:], a, func, bias=bias, scale=scale)
        return o[:]

    def recip(self, a):
        o = self.t()
        self.nc.vector.reciprocal(o[:], a)
        return o[:]


def _emit_final_math(nc, fm, st_rxy, st_g, sxp, syp, n_ap, out_ap):
    """st_rxy: [128, 9K] raw moments, col (3i+j)K+k; st_g: [128, K] (Gx+Gy);
    sxp/syp: [128, 3K] packed raw sums, col iK+k; n_ap: [128, K]."""
    K = fm.K
    pool_eng = nc.gpsimd
    rn = fm.recip(n_ap)

    # C = st_rxy - (Sx_i Sy_j) rn : three wide [128, 9K] ops
    sxA = sxp.rearrange("p (i k) -> p i k", i=3).unsqueeze(2).broadcast_to([ROWS, 3, 3, K])
    syB = syp.rearrange("p (j k) -> p j k", j=3).unsqueeze(1).broadcast_to([ROWS, 3, 3, K])
    t1 = fm.t(9 * K)
    nc.vector.tensor_tensor(t1[:].rearrange("p (i j k) -> p i j k", i=3, j=3), sxA, syB, OP.mult)
    rn9 = rn.unsqueeze(1).broadcast_to([ROWS, 9, K])
    t2 = fm.t(9 * K)
    nc.vector.tensor_tensor(t2[:].rearrange("p (g k) -> p g k", g=9), t1[:].rearrange("p (g k) -> p g k", g=9), rn9, OP.mult)
    Ct = fm.sub(st_rxy, t2[:], w=9 * K)
    Cr = Ct.rearrange("p (i j k) -> p i j k", i=3, j=3)

    def C(i, j):
        return Ct[:, (3 * i + j) * K : (3 * i + j + 1) * K]

    # gxy = (Gx + Gy raw, pre-merged in st_g) - (|Sx|^2 + |Sy|^2) rn  (Pool)
    sqx = fm.mul(sxp, sxp, w=3 * K, eng=pool_eng)
    sqy = fm.mul(syp, syp, w=3 * K, eng=pool_eng)
    sq1 = fm.add(sqx, sqy, w=3 * K, eng=pool_eng)
    sq2 = fm.add(sq1[:, 0:K], sq1[:, K : 2 * K], eng=pool_eng)
    sq3 = fm.add(sq2, sq1[:, 2 * K : 3 * K], eng=pool_eng)
    sqrn = fm.mul(sq3, rn, eng=pool_eng)
    gxy = fm.sub(st_g, sqrn, eng=pool_eng)

    # K = C^T C packed into kkp [128, 6K]: order [00, 11, 22, 01, 02, 12]
    kkp = fm.t(6 * K)
    for idx, (a, b) in enumerate([(0, 0), (1, 1), (2, 2), (0, 1), (0, 2), (1, 2)]):
        m = fm.t(3 * K)
        nc.vector.tensor_tensor(
            m[:].rearrange("p (i k) -> p i k", i=3), Cr[:, :, a, :], Cr[:, :, b, :], OP.mult
        )
        f1 = fm.add(m[:, 0:K], m[:, K : 2 * K])
        nc.vector.tensor_tensor(
            kkp[:, idx * K : (idx + 1) * K], f1, m[:, 2 * K : 3 * K], OP.add
        )
    kdiag = kkp[:, 0 : 3 * K]
    koff = kkp[:, 3 * K : 6 * K]
    k01 = kkp[:, 3 * K : 4 * K]
    k02 = kkp[:, 4 * K : 5 * K]
    k12 = kkp[:, 5 * K : 6 * K]

    # det(C)  (Pool branch; only feeds the Kabsch sign)
    m0 = fm.sub(fm.mul(C(1, 1), C(2, 2), eng=pool_eng), fm.mul(C(1, 2), C(2, 1), eng=pool_eng), eng=pool_eng)
    m1 = fm.sub(fm.mul(C(1, 0), C(2, 2), eng=pool_eng), fm.mul(C(1, 2), C(2, 0), eng=pool_eng), eng=pool_eng)
    m2 = fm.sub(fm.mul(C(1, 0), C(2, 1), eng=pool_eng), fm.mul(C(1, 1), C(2, 0), eng=pool_eng), eng=pool_eng)
    d0 = fm.mul(C(0, 0), m0, eng=pool_eng)
    d1 = fm.mul(C(0, 1), m1, eng=pool_eng)
    d2 = fm.mul(C(0, 2), m2, eng=pool_eng)
    detC = fm.add(fm.sub(d0, d1, eng=pool_eng), d2, eng=pool_eng)
    # Kabsch sign d = 1 - 2*(detC < 0)  (stay on Pool-adjacent DVE ops)
    neg = fm.ts(detC, 0.0, OP.is_lt)
    dsg = fm.ts2(neg, -2.0, 1.0, OP.mult, OP.add)

    # q = tr(K)/3 ; kd = diag - q (one wide op); p2 = |kd|^2 + 2 |koff|^2
    trk = fm.add(fm.add(kkp[:, 0:K], kkp[:, K : 2 * K]), kkp[:, 2 * K : 3 * K])
    q = fm.ts(trk, 1.0 / 3.0, OP.mult)
    kdp = fm.t(3 * K)
    q3 = q.unsqueeze(1).broadcast_to([ROWS, 3, K])
    nc.vector.tensor_tensor(
        kdp[:].rearrange("p (i k) -> p i k", i=3),
        kdiag.rearrange("p (i k) -> p i k", i=3), q3, OP.subtract,
    )
    sq_off = fm.mul(koff, koff, w=3 * K)
    kdsq = fm.mul(kdp[:], kdp[:], w=3 * K)
    p2a = fm.add(fm.add(kdsq[:, 0:K], kdsq[:, K : 2 * K]), kdsq[:, 2 * K : 3 * K])
    xsq = fm.add(
        fm.add(sq_off[:, 0:K], sq_off[:, K : 2 * K]), sq_off[:, 2 * K : 3 * K]
    )
    p2 = fm.stt(xsq, 2.0, p2a, OP.mult, OP.add)  # p2a + 2*xsq
    p2c = fm.ts2(p2, 1.0 / 6.0, 1e-30, OP.mult, OP.max)
    p = fm.act(p2c, AF.Sqrt)

    # det(K - qI) (symmetric)
    e0 = fm.mul(kdp[:, 0:K], fm.sub(fm.mul(kdp[:, K : 2 * K], kdp[:, 2 * K : 3 * K]), sq_off[:, 2 * K : 3 * K]))
    e1 = fm.mul(k01, fm.sub(fm.mul(k01, kdp[:, 2 * K : 3 * K]), fm.mul(k12, k02)))
    e2 = fm.mul(k02, fm.sub(fm.mul(k01, k12), fm.mul(kdp[:, K : 2 * K], k02)))
    detKq = fm.add(fm.sub(e0, e1), e2)

    # r = 0.5 det(K-qI) / p^3, clamped to [-1, 1]
    rp = fm.recip(p)
    r = fm.mul(fm.mul(fm.ts(detKq, 0.5, OP.mult), rp), fm.mul(rp, rp))
    r = fm.ts2(r, 1.0, -1.0, OP.min, OP.max)

    # acos via |r| fold:
    #   A = 2 atan(sqrt((1-|r|)/(1+|r|))) = acos(|r|)
    #   acos(r) = A + (r<0) * (pi - 2A) ; phi = acos(r)/3
    rabs = fm.stt(r, -1.0, r, OP.mult, OP.max)  # |r| = max(-r, r)
    onemr = fm.ts2(rabs, -1.0, 1.0, OP.mult, OP.add)  # 1 - |r|
    onepr = fm.ts(rabs, 1.0, OP.add)
    u = fm.mul(onemr, fm.recip(onepr))
    su = fm.act(u, AF.Sqrt)
    at = fm.act(su, AF.Arctan)  # Act->Act, single round trip
    A = fm.ts(at, 2.0, OP.mult)
    rneg = fm.ts(r, 0.0, OP.is_lt)
    corr = fm.ts2(A, -2.0, float(np.pi), OP.mult, OP.add)  # pi - 2A
    acr = fm.add(A, fm.mul(rneg, corr))
    # cos(phi) = sin(phi/1 + pi/2) ; cos(phi + 2pi/3) = -sin(5pi/6 - phi)
    # pack both sin args into one [128, 2K] tile -> single Act Sin
    sargs = fm.t(2 * K)
    nc.vector.tensor_scalar(sargs[:, 0:K], acr, 1.0 / 3.0, float(np.pi / 2), OP.mult, OP.add)
    nc.vector.tensor_scalar(sargs[:, K : 2 * K], acr, -1.0 / 3.0, float(5 * np.pi / 6), OP.mult, OP.add)
    sc = fm.act(sargs[:], AF.Sin, w=2 * K)

    # eigenvalues packed: l = [l1 | l2 | l3], one clamp + one sqrt
    p2x = fm.ts(p, 2.0, OP.mult)
    lp = fm.t(3 * K)
    l1 = fm.add(q, fm.mul(p2x, sc[:, 0:K]))
    l3 = fm.sub(q, fm.mul(p2x, sc[:, K : 2 * K]))
    nc.vector.tensor_copy(lp[:, 0:K], l1)
    nc.vector.tensor_copy(lp[:, 2 * K : 3 * K], l3)
    nc.vector.tensor_tensor(
        lp[:, K : 2 * K], fm.stt(q, 3.0, l1, OP.mult, OP.subtract), l3, OP.subtract
    )
    lc = fm.ts(lp[:], 0.0, OP.max, w=3 * K)
    sv = fm.act(lc, AF.Sqrt, w=3 * K)

    tr = fm.add(fm.add(sv[:, 0:K], sv[:, K : 2 * K]), fm.mul(dsg, sv[:, 2 * K : 3 * K]))

    # msd = (gxy - 2 tr) rn ; rmsd = sqrt(max(msd, 0)) written straight out
    diff = fm.stt(tr, -2.0, gxy, OP.mult, OP.add)
    msd = fm.mul(diff, rn)
    nc.scalar.activation(out_ap, fm.ts(msd, 0.0, OP.max), AF.Sqrt, bias=0.0, scale=1.0)


# ---------------------------------------------------------------------------
# Program builder
# ---------------------------------------------------------------------------
def build_program(caps, nmax, cfg=None):
    """caps: per-class atom capacities (multiples of 256). Returns nc."""
    cfg = cfg or {}
    K = len(caps)
    chunks = [c // CHUNK for c in caps]
    assert all(c * CHUNK == cap for c, cap in zip(chunks, caps))
    offs = np.cumsum([0] + [c * CW for c in chunks])
    TOTW = int(offs[-1])

    install_tile_patch()
    nc = bass.Bass()
    x_d = nc.dram_tensor("x", [ROWS, TOTW], FP8, kind="ExternalInput")
    y_d = nc.dram_tensor("y", [ROWS, TOTW], FP8, kind="ExternalInput")
    meta_d = nc.dram_tensor("meta", [ROWS, K], F32, kind="ExternalInput")
    # masks packed bf16: [imask (128) | gmask (780)]
    MSKW = ROWS + 2 * GW
    masks_d = nc.dram_tensor("masks", [ROWS, MSKW], BF16, kind="ExternalInput")
    out_d = nc.dram_tensor("out", [ROWS, K], F32, kind="ExternalOutput")

    with TileContext(nc) as tc:
        with (
            tc.tile_pool(name="const", bufs=1) as constp,
            tc.tile_pool(name="data", bufs=1) as datap,
            tc.tile_pool(name="stats", bufs=1) as statp,
            tc.tile_pool(name="cps", bufs=2) as cpsp,
            tc.psum_pool(name="pj", bufs=2) as pjp,
            tc.psum_pool(name="pg", bufs=1) as pgp,
        ):
            meta_t_tile = constp.tile([ROWS, K], F32)
            masks_t = constp.tile([ROWS, MSKW], BF16)
            meta_t = meta_t_tile[:]
            imask_t = masks_t[:, 0:ROWS]
            gmask_t = masks_t[:, ROWS:MSKW]

            # DMA order: first two classes' coords, then masks (needed by the
            # first extraction), remaining classes, meta last. Big classes are
            # split in half so PE trails arrivals at sub-class granularity.
            korder = list(cfg.get("korder", range(K)))

            def class_dmas(k):
                a, b = int(offs[k]), int(offs[k + 1])
                nck = chunks[k]
                parts = []
                if nck >= 4:
                    mid = a + (nck // 2) * CW
                    parts = [(a, mid), (mid, b)]
                else:
                    parts = [(a, b)]
                for (p0, p1) in parts:
                    nc.sync.dma_start(out=x_t[:, p0:p1], in_=x_d[:, p0:p1])
                    nc.sync.dma_start(out=y_t[:, p0:p1], in_=y_d[:, p0:p1])

            x_t = datap.tile([ROWS, TOTW], FP8, name="x_t", tag="x_t")
            y_t = datap.tile([ROWS, TOTW], FP8, name="y_t", tag="y_t")
            for k in korder[:2]:
                class_dmas(k)
            nc.sync.dma_start(out=masks_t[:], in_=masks_d[:])
            for k in korder[2:]:
                class_dmas(k)
            nc.sync.dma_start(out=meta_t_tile[:], in_=meta_d[:])

            st_rxy = statp.tile([ROWS, 9 * K], F32)
            st_g = statp.tile([ROWS, K], F32)
            sxp = statp.tile([ROWS, 3 * K], F32, name="sxp", tag="sxp")
            syp = statp.tile([ROWS, 3 * K], F32, name="syp", tag="syp")
            # Rotating scratch tiles: extraction accums are independent, a
            # single scratch would serialize them through WAW hazards.
            scrs = [
                statp.tile([ROWS, 2 * GW], F32, name=f"scr{i}", tag=f"scr{i}")
                for i in range(4)
            ]
            _scr_n = [0]

            def scr_next():
                t = scrs[_scr_n[0] % len(scrs)]
                _scr_n[0] += 1
                return t

            phase = cfg.get("phase", "full")  # dma|mm|copies|extract|full

            for k in korder:
                if phase == "dma":
                    break
                base_k = int(offs[k])
                nck = chunks[k]
                oj = [
                    pjp.tile([ROWS, JW], F32, name=f"oj{j}", tag=f"oj{j}")
                    for j in range(3)
                ]
                ogx = pgp.tile([ROWS, GW], F32, name="ogx", tag="ogx")
                ogy = pgp.tile([ROWS, GW], F32, name="ogy", tag="ogy")
                # Multiple accumulation groups share the ogx/ogy banks and
                # start=True zeroes a whole bank, so only the very first
                # matmul of each bank carries start=True.

                # Group-major matmul order: each gram's accumulation group
                # finishes as early as possible so its copy/extraction can
                # start while later groups still run on PE.
                def rhs_all(cb):
                    return x_t[:, cb : cb + CW].rearrange(
                        "p (b c2 two) -> p two b c2", two=2, b=3
                    )[:, :, :, ::-1]

                def rhs_blk(t, cb, i):
                    sl = slice(cb + BLK * i, cb + BLK * (i + 1))
                    return t[:, sl].rearrange("p (c2 two) -> p two c2", two=2)[:, :, ::-1]

                cj = [
                    cpsp.tile([ROWS, JW], F32, name=f"cj{j}", tag=f"cj{j}")
                    for j in range(3)
                ]
                cg = cpsp.tile([ROWS, 2 * GW], F32, name="cg", tag="cg")
                do_ce = phase not in ("mm",)

                for i in range(3):
                    for c in range(nck):
                        cb = base_k + c * CW
                        nc.tensor.matmul(
                            ogx[:, 130 * i : 130 * i + 129],
                            x_t[:, cb + BLK * i + 2 : cb + BLK * (i + 1)],
                            rhs_blk(x_t, cb, i),
                            start=(c == 0 and i == 0), stop=(c == nck - 1),
                            perf_mode=MM.DoubleRowSwInterleave,
                            skip_group_check=True,
                        )
                for j in range(3):
                    for c in range(nck):
                        cb = base_k + c * CW
                        nc.tensor.matmul(
                            ogy[:, 130 * j : 130 * j + 129],
                            y_t[:, cb + BLK * j + 2 : cb + BLK * (j + 1)],
                            rhs_blk(y_t, cb, j),
                            start=(c == 0 and j == 0), stop=(c == nck - 1),
                            perf_mode=MM.DoubleRowSwInterleave,
                            skip_group_check=True,
                        )
                if do_ce:
                    nc.scalar.copy(cg[:, 0:GW], ogx[:])
                    nc.scalar.copy(cg[:, GW : 2 * GW], ogy[:])
                for j in range(3):
                    for c in range(nck):
                        cb = base_k + c * CW
                        nc.tensor.matmul(
                            oj[j][:],
                            y_t[:, cb + BLK * j + 2 : cb + BLK * (j + 1)],
                            rhs_all(cb),
                            start=(c == 0), stop=(c == nck - 1),
                            perf_mode=MM.DoubleRowSwInterleave,
                        )
                    if do_ce:
                        nc.scalar.copy(cj[j][:], oj[j][:])

                if phase in ("mm", "copies"):
                    continue
                # DVE: masked diagonal extraction
                # st_rxy layout: col (3i+j)*K + k ; st_g: Gx+Gy at col k
                nc.vector.scalar_tensor_tensor(
                    scr_next()[:], cg[:], 1.0, gmask_t, OP.mult, OP.mult,
                    accum_out=st_g[:, k : k + 1],
                )
                # Raw sums from the ones-columns (cols 128::130 of each bank):
                # sxp/syp [128, 3K], col i*K + k.
                nc.vector.tensor_copy(sxp[:, k : 2 * K + k + 1 : K], cg[:, 128:GW:130])
                nc.vector.tensor_copy(syp[:, k : 2 * K + k + 1 : K], cg[:, GW + 128 : 2 * GW : 130])
                for j in range(3):
                    for i in range(3):
                        col = (3 * i + j) * K + k
                        nc.vector.scalar_tensor_tensor(
                            scr_next()[:, 0:ROWS],
                            cj[j][:, 129 * i : 129 * i + ROWS],
                            1.0, imask_t, OP.mult, OP.mult,
                            accum_out=st_rxy[:, col : col + 1],
                        )

            out_t = statp.tile([ROWS, K], F32)
            if phase == "full":
                fm = _FM(nc, statp, K)
                _emit_final_math(
                    nc, fm, st_rxy[:], st_g[:], sxp[:], syp[:], meta_t, out_t[:]
                )
            else:
                nc.vector.memset(out_t[:], 0.0)
            nc.sync.dma_start(out=out_d[:], in_=out_t[:])

            if cfg.get("debug"):
                dbg_rxy = nc.dram_tensor("dbg_rxy", [ROWS, 9 * K], F32, kind="ExternalOutput")
                dbg_g = nc.dram_tensor("dbg_g", [ROWS, K], F32, kind="ExternalOutput")
                dbg_sx = nc.dram_tensor("dbg_sx", [ROWS, 3 * K], F32, kind="ExternalOutput")
                dbg_sy = nc.dram_tensor("dbg_sy", [ROWS, 3 * K], F32, kind="ExternalOutput")
                nc.sync.dma_start(out=dbg_rxy[:], in_=st_rxy[:])
                nc.sync.dma_start(out=dbg_g[:], in_=st_g[:])
                nc.sync.dma_start(out=dbg_sx[:], in_=sxp[:])
                nc.sync.dma_start(out=dbg_sy[:], in_=syp[:])

    return nc


# ---------------------------------------------------------------------------
# Host side
# ---------------------------------------------------------------------------
def plan_shards(num_atoms, n_classes=4, cap_round=CHUNK):
    B = num_atoms.shape[0]
    assert B % (N_CORES * ROWS) == 0
    n_classes_total = B // (N_CORES * ROWS)
    assert n_classes == n_classes_total
    order = np.argsort(num_atoms, kind="stable")
    na_sorted = num_atoms[order]
    rows_per_class = N_CORES * ROWS
    caps = []
    for k in range(n_classes):
        mx = int(na_sorted[(k + 1) * rows_per_class - 1])
        cap = ((mx + cap_round - 1) // cap_round) * cap_round
        caps.append(cap)
    return order, caps


def shard_inputs(coords_input, coords_target, num_atoms, order, caps, nmax):
    import ml_dtypes

    K = len(caps)
    rows_per_class = N_CORES * ROWS
    chunks = [c // CHUNK for c in caps]
    TOTW = sum(c * CW for c in chunks)

    import ml_dtypes as _mld

    imask = np.eye(ROWS, dtype=np.float32)
    gmask = np.zeros((ROWS, 2 * GW), dtype=np.float32)
    r = np.arange(ROWS)
    for i in range(3):
        gmask[r, 130 * i + r] = 1.0
        gmask[r, GW + 130 * i + r] = 1.0
    masks = np.concatenate([imask, gmask], axis=1).astype(_mld.bfloat16)

    in_maps = []
    core_row_idx = []
    for c in range(N_CORES):
        idx = np.concatenate(
            [
                order[k * rows_per_class + c * ROWS : k * rows_per_class + (c + 1) * ROWS]
                for k in range(K)
            ]
        )
        core_row_idx.append(idx)
        na_c = num_atoms[idx].astype(np.int64)  # [K*ROWS]

        bufs = {"x": np.zeros((ROWS, TOTW), dtype=ml_dtypes.float8_e4m3fn),
                "y": np.zeros((ROWS, TOTW), dtype=ml_dtypes.float8_e4m3fn)}
        off = 0
        for k in range(K):
            rows_k = idx[k * ROWS : (k + 1) * ROWS]
            na_k = na_c[k * ROWS : (k + 1) * ROWS]
            cap = caps[k]
            nck = chunks[k]
            amask = (np.arange(cap)[None, :] < na_k[:, None])
            for nm, coords in (("x", coords_input), ("y", coords_target)):
                v = coords[rows_k, : 3 * cap].reshape(ROWS, cap, 3)
                v = np.where(amask[:, :, None], v, 0.0).astype(np.float32)
                # [r, ch, t, p, i] -> [p, ch, i, r, t]
                v = v.reshape(ROWS, nck, 2, 128, 3).transpose(3, 1, 4, 0, 2)
                blk = np.zeros((128, nck, 3, 129, 2), dtype=ml_dtypes.float8_e4m3fn)
                blk[:, :, :, 1:, :] = v[:, :, :, ::-1, :].astype(
                    ml_dtypes.float8_e4m3fn
                )
                blk[:, :, :, 0, :] = 1.0
                w = nck * CW
                bufs[nm][:, off : off + w] = blk.reshape(128, w)
            off += nck * CW

        meta = np.ascontiguousarray(na_c.astype(np.float32).reshape(K, ROWS).T)
        in_maps.append({"x": bufs["x"], "y": bufs["y"], "meta": meta, "masks": masks})
    return in_maps, core_row_idx


def unshard_outputs(results, core_row_idx, B):
    out = np.empty(B, dtype=np.float32)
    for c in range(N_CORES):
        o = results[c]["out"]  # [ROWS, K]
        idx = core_row_idx[c]
        out[idx] = o.T.reshape(-1)
    return out


# ---------------------------------------------------------------------------
# Entry point: full inputs in, full output out. Shards across 8 NeuronCores.
# ---------------------------------------------------------------------------
_PROG_CACHE = {}


def _get_program(caps, nmax):
    key = (tuple(caps), nmax)
    if key not in _PROG_CACHE:
        _PROG_CACHE[key] = build_program(list(caps), nmax)
    return _PROG_CACHE[key]


def kernel(coords_input, coords_target, num_atoms):
    from concourse.bass_utils import run_bass_kernel_spmd

    x = np.ascontiguousarray(np.asarray(coords_input, dtype=np.float32))
    y = np.ascontiguousarray(np.asarray(coords_target, dtype=np.float32))
    na = np.asarray(num_atoms)
    na_i = na.astype(np.int64)
    B, ncols = x.shape
    nmax = ncols // 3
    K = B // (N_CORES * ROWS)
    assert B == N_CORES * ROWS * K, f"unsupported batch {B}"

    order, caps = plan_shards(na_i, n_classes=K)
    in_maps, core_row_idx = shard_inputs(x, y, na_i, order, caps, nmax)
    nc = _get_program(caps, nmax)
    res = run_bass_kernel_spmd(nc, in_maps, core_ids=list(range(N_CORES)))
    out = unshard_outputs(res.results, core_row_idx, B)
    return out.astype(np.float32)
